# revision 4
# baseline (speedup 1.0000x reference)
"""Trainium2 Bass kernel: per-pixel 5x5-patch channel covariance.

R[b,h,w,k,l] = (1/N) sum_n (p_kn - mu_k)(p_ln - mu_l)   (N=25, reflect pad)

Identity:  R = box5x5(S_k * S_l)/25 - mu_k * mu_l,  mu = box5x5(S)/25.
Separable box sums run as banded matmuls on TensorE; reflect padding is
folded into the band weights. Host pre-scales S by 1/5 so the two band
passes produce box/25 directly.

v2: only the 136 upper-triangle pair channels are computed/DMA'd (pair-major
order); the host mirrors to the full 16x16. Stage-2 psum is consumed by
packed-layout subtracts; work is split across DVE / Act / GpSimd. Psum
tiles span two channel octets so evac/sub instructions are large.

Sharding: 8 cores = 4 batches x 2 H-halves. Fully data parallel.
"""
import sys

sys.path.insert(0, "/opt/trn_rl_repo")

from contextlib import ExitStack

import numpy as np

import concourse.bacc as bacc
import concourse.mybir as mybir
import concourse.tile as tile
from concourse import bass_utils

B, K, H, W = 4, 16, 256, 256
HH = 128           # output rows per core
SR = 132           # shard rows (128 + 2 halo each side, edge-clamped)
NP = 136           # upper-triangle pairs, k-major: (0,0)..(0,15),(1,1)..
NCH = K + NP       # 152 channels: 0..15 mean, 16.. pairs (pr order)
NOCT = NCH // 8    # 19 channel octets (oct 0,1 = means; 2..18 = pairs)
F32 = mybir.dt.float32
BF16 = mybir.dt.bfloat16

# ---- tuning knobs (engine routing) ----
D_POOL_FRAC = 3    # every Nth sub unit routed Act-evac + GpSimd-sub
C_POOL_KMAX = 3    # M k-runs with k < KMAX go to GpSimd, rest DVE


def _reflect_idx(i, n):
    if i < 0:
        return -i
    if i >= n:
        return 2 * (n - 1) - i
    return i


def _build_bw():
    """[256 w'col, 256 wout] box weights with reflection folded; -> [128, 4*128]
    blocks indexed (oh, chunk): BW[:, (oh*2+c)*128 + wl] = M[c*128 + :, oh*128 + wl]."""
    M = np.zeros((W, W), dtype=np.float32)
    for w in range(W):
        for j in range(5):
            M[_reflect_idx(w - 2 + j, W), w] += 1.0
    out = np.zeros((128, 512), dtype=np.float32)
    for oh in range(2):
        for c in range(2):
            out[:, (oh * 2 + c) * 128:(oh * 2 + c) * 128 + 128] = \
                M[c * 128:(c + 1) * 128, oh * 128:(oh + 1) * 128]
    return out


def _build_br(half):
    """[68, 128]: cols rt*64+hl; rows are shard-local rows within row-tile rt."""
    hbase = half * HH
    M = np.zeros((68, 128), dtype=np.float32)
    for rt in range(2):
        for hl in range(64):
            hg = hbase + rt * 64 + hl
            for i in range(5):
                r = _reflect_idx(hg - 2 + i, H)
                j = r + 2 - hbase          # canonical shard row
                M[j - rt * 64, rt * 64 + hl] += 1.0
    return M


def _ksegs_in_octet(oct_idx):
    """Pair channels live at ch 16..151 (pr k-major). For octet [oct*8, +8),
    return (j0, k, l0, nl): local offset j0, channel k, first l, count."""
    lo, hi = oct_idx * 8, oct_idx * 8 + 8
    segs = []
    p = 0
    for k in range(K):
        n = K - k
        s, e = 16 + p, 16 + p + n
        a, b = max(lo, s), min(hi, e)
        if a < b:
            segs.append((a - lo, k, k + (a - s), b - a))
        p += n
    return segs


def _pr0(k):
    """pr index of pair (k, k)."""
    return k * K - (k * (k - 1)) // 2


def _build_kernel():
    nc = bacc.Bacc("TRN2", target_bir_lowering=False, debug=False)
    S_d = nc.dram_tensor("S", [SR, K, W], BF16, kind="ExternalInput").ap()
    BR_d = nc.dram_tensor("BR", [68, 128], BF16, kind="ExternalInput").ap()
    BW_d = nc.dram_tensor("BW", [128, 512], BF16, kind="ExternalInput").ap()
    # output: upper triangle only, [w, rt, pr, hh] bf16
    R_d = nc.dram_tensor("R", [W, 2, NP, 64], BF16, kind="ExternalOutput").ap()

    with tile.TileContext(nc) as tc, ExitStack() as ctx:
        const_p = ctx.enter_context(tc.tile_pool(name="const", bufs=1))
        sp_p = ctx.enter_context(tc.tile_pool(name="sp", bufs=1))
        t_p = ctx.enter_context(tc.tile_pool(name="tprod", bufs=3))
        i1_p = ctx.enter_context(tc.tile_pool(name="i1", bufs=2))
        mu_p = ctx.enter_context(tc.tile_pool(name="mu", bufs=2))
        m_p = ctx.enter_context(tc.tile_pool(name="mm", bufs=2))
        r_p = ctx.enter_context(tc.tile_pool(name="rout", bufs=2))
        e2_p = ctx.enter_context(tc.tile_pool(name="e2", bufs=3))
        ps1_p = ctx.enter_context(tc.tile_pool(name="ps1", bufs=2, space="PSUM"))
        ps2_p = ctx.enter_context(tc.tile_pool(name="ps2", bufs=2, space="PSUM"))

        br = const_p.tile([68, 128], BF16)
        bw = const_p.tile([128, 512], BF16)
        nc.sync.dma_start(br[:], BR_d)
        nc.sync.dma_start(bw[:], BW_d)

        sp0 = sp_p.tile([68, K, W], BF16)
        sp1 = sp_p.tile([68, K, W], BF16)
        nc.sync.dma_start(sp0[:], S_d[0:68])
        nc.sync.dma_start(sp1[:], S_d[64:132])
        sps = [sp0, sp1]

        # ---------------- stage 1: products + H-box ----------------
        i1s = []
        for rt in range(2):
            sp = sps[rt]
            brt = br[:, rt * 64:(rt + 1) * 64]
            # i1 layout: [w(128), wchunk(2), ch(NCH), h(64)]
            i1 = i1_p.tile([128, 2, NCH, 64], BF16, name="i1")
            i1s.append(i1)
            for oc in range(NOCT):
                if oc < 2:   # mean channels read straight from sp
                    T = sp[:, oc * 8:(oc + 1) * 8, :]
                else:
                    Tt = t_p.tile([68, 8, W], BF16, name="T")
                    for (j0, k, l0, nl) in _ksegs_in_octet(oc):
                        in0 = sp[:, k, :].unsqueeze(1).broadcast_to([68, nl, W])
                        nc.vector.tensor_mul(
                            Tt[:, j0:j0 + nl, :], in0, sp[:, l0:l0 + nl, :])
                    T = Tt[:]
                ps1 = ps1_p.tile([128, 2, 8, 64], F32, name="ps1")
                for j in range(8):
                    nc.tensor.matmul(ps1[:, 0, j, :],
                                     T[:, j, 0:128], brt,
                                     start=True, stop=True)
                    nc.tensor.matmul(ps1[:, 1, j, :],
                                     T[:, j, 128:256], brt,
                                     start=True, stop=True)
                nc.scalar.copy(i1[:, :, oc * 8:(oc + 1) * 8, :], ps1[:])

        # ---------------- stage 2: W-box + finish ----------------
        for rt in range(2):
            i1 = i1s[rt]
            for oh in range(2):
                bwa = bw[:, (oh * 2) * 128:(oh * 2) * 128 + 128]
                bwb = bw[:, (oh * 2 + 1) * 128:(oh * 2 + 1) * 128 + 128]
                # --- mean channels -> mu (bf16, scaled by 1/5) ---
                mub = mu_p.tile([128, K, 64], BF16, name="mub")
                psm = ps2_p.tile([128, 2, 8, 64], F32, name="ps2")
                for mo in range(2):
                    nc.tensor.matmul(psm[:, mo], bwa,
                                     i1[:, 0, mo * 8:(mo + 1) * 8, :],
                                     start=True, stop=False)
                    nc.tensor.matmul(psm[:, mo], bwb,
                                     i1[:, 1, mo * 8:(mo + 1) * 8, :],
                                     start=False, stop=True)
                nc.scalar.mul(mub[:].rearrange("p a b -> p (a b)"),
                              psm[:].rearrange("p a b c -> p (a b c)"), 0.2)
                # --- M = mu_k * mu_l (pr-major), DVE/GpSimd split ---
                M = m_p.tile([128, NP, 64], BF16, name="M")
                for k in range(K):
                    nl = K - k
                    p0 = _pr0(k)
                    in0 = mub[:, k, :].unsqueeze(1).broadcast_to([128, nl, 64])
                    eng = nc.gpsimd if k < C_POOL_KMAX else nc.vector
                    eng.tensor_mul(M[:, p0:p0 + nl, :], in0, mub[:, k:K, :])
                # --- pair channels: W-box, subtract, store triangle ---
                rsb = r_p.tile([128, NP, 64], BF16, name="rsb")
                unit = 0
                oc = 2
                while oc < NOCT:
                    no = min(2, NOCT - oc)     # octets in this psum tile
                    ps2 = ps2_p.tile([128, 2, 8, 64], F32, name="ps2")
                    for i in range(no):
                        nc.tensor.matmul(ps2[:, i], bwa,
                                         i1[:, 0, (oc + i) * 8:(oc + i + 1) * 8, :],
                                         start=True, stop=False)
                        nc.tensor.matmul(ps2[:, i], bwb,
                                         i1[:, 1, (oc + i) * 8:(oc + i + 1) * 8, :],
                                         start=False, stop=True)
                    pr = (oc - 2) * 8
                    npr = no * 8
                    p2v = ps2[:, 0:no].rearrange("p a b c -> p (a b) c")
                    gunit = (rt * 2 + oh) * 9 + unit
                    if gunit % D_POOL_FRAC == D_POOL_FRAC - 1:
                        e2 = e2_p.tile([128, 16, 64], BF16, name="e2")
                        nc.scalar.copy(e2[:, 0:npr, :], p2v)
                        nc.gpsimd.tensor_sub(rsb[:, pr:pr + npr, :],
                                             e2[:, 0:npr, :],
                                             M[:, pr:pr + npr, :])
                    else:
                        nc.vector.tensor_sub(rsb[:, pr:pr + npr, :], p2v,
                                             M[:, pr:pr + npr, :])
                    oc += no
                    unit += 1
                # DMA out: R[w-half oh, rt, :, :]
                nc.sync.dma_start(R_d[oh * 128:(oh + 1) * 128, rt], rsb[:])

    nc.compile()
    return nc


_NC_CACHE = {}


def _get_nc():
    if "nc" not in _NC_CACHE:
        _NC_CACHE["nc"] = _build_kernel()
    return _NC_CACHE["nc"]


def _prep_in_maps(S):
    S = np.asarray(S, dtype=np.float32)
    np_bf16 = mybir.dt.np(BF16)
    bw = _build_bw().astype(np_bf16)
    brs = [(_build_br(h)).astype(np_bf16) for h in range(2)]
    Ss = S * np.float32(0.2)
    in_maps = []
    for b in range(B):
        for half in range(2):
            hbase = half * HH
            rows = np.clip(np.arange(hbase - 2, hbase + 130), 0, H - 1)
            shard = Ss[b][:, rows, :].transpose(1, 0, 2)   # [132, K, 256]
            shard = np.ascontiguousarray(shard).astype(np_bf16)
            in_maps.append({"S": shard, "BR": brs[half], "BW": bw})
    return in_maps


# upper-tri gather index: IU[k, l] = pr for (min,max)
_IU = np.zeros((K, K), dtype=np.int64)
for _k in range(K):
    for _l in range(_k, K):
        _IU[_k, _l] = _IU[_l, _k] = _pr0(_k) + (_l - _k)


def _assemble(results):
    out = np.empty((B, H, W, K, K), dtype=np.float32)
    for i in range(8):
        b, half = divmod(i, 2)
        rd = np.asarray(results[i]["R"]).astype(np.float32)  # [256, 2, 136, 64]
        tri = rd.transpose(1, 3, 0, 2).reshape(HH, W, NP)    # [h, w, pr]
        out[b, half * HH:(half + 1) * HH] = tri[:, :, _IU]
    return out


def kernel(S):
    """S: [4, 16, 256, 256] float32 -> R: [4, 256, 256, 16, 16] float32."""
    nc = _get_nc()
    in_maps = _prep_in_maps(S)
    res = bass_utils.run_bass_kernel_spmd(nc, in_maps, list(range(8)))
    return _assemble(res.results)


# revision 6
# speedup vs baseline: 1.2117x; 1.2117x over previous
"""Trainium2 Bass kernel: per-pixel 5x5-patch channel covariance.

R[b,h,w,k,l] = (1/N) sum_n (p_kn - mu_k)(p_ln - mu_l)   (N=25, reflect pad)

Identity:  R = box5x5(S_k * S_l)/25 - mu_k * mu_l,  mu = box5x5(S)/25.
Separable box sums run as banded matmuls on TensorE; reflect padding is
folded into the band weights. Host pre-scales S by 1/5 so the two band
passes produce box/25 directly.

v2: only the 136 upper-triangle pair channels are computed/DMA'd (pair-major
order); the host mirrors to the full 16x16. Stage-2 psum is consumed by
packed-layout subtracts; work is split across DVE / Act / GpSimd. Psum
tiles span two channel octets so evac/sub instructions are large.

Sharding: 8 cores = 4 batches x 2 H-halves. Fully data parallel.
"""
import sys

sys.path.insert(0, "/opt/trn_rl_repo")

from contextlib import ExitStack

import numpy as np

import concourse.bacc as bacc
import concourse.mybir as mybir
import concourse.tile as tile
from concourse import bass_utils

B, K, H, W = 4, 16, 256, 256
HH = 128           # output rows per core
SR = 132           # shard rows (128 + 2 halo each side, edge-clamped)
NP = 136           # upper-triangle pairs, k-major: (0,0)..(0,15),(1,1)..
NCH = K + NP       # 152 channels: 0..15 mean, 16.. pairs (pr order)
NOCT = NCH // 8    # 19 channel octets (oct 0,1 = means; 2..18 = pairs)
F32 = mybir.dt.float32
BF16 = mybir.dt.bfloat16

# ---- tuning knobs (engine routing) ----
D_POOL_FRAC = 3    # every Nth sub unit routed Act-evac + GpSimd-sub
C_POOL_KMAX = 3    # M k-runs with k < KMAX go to GpSimd, rest DVE


def _reflect_idx(i, n):
    if i < 0:
        return -i
    if i >= n:
        return 2 * (n - 1) - i
    return i


def _build_bw():
    """[256 w'col, 256 wout] box weights with reflection folded; -> [128, 4*128]
    blocks indexed (oh, chunk): BW[:, (oh*2+c)*128 + wl] = M[c*128 + :, oh*128 + wl]."""
    M = np.zeros((W, W), dtype=np.float32)
    for w in range(W):
        for j in range(5):
            M[_reflect_idx(w - 2 + j, W), w] += 1.0
    out = np.zeros((128, 512), dtype=np.float32)
    for oh in range(2):
        for c in range(2):
            out[:, (oh * 2 + c) * 128:(oh * 2 + c) * 128 + 128] = \
                M[c * 128:(c + 1) * 128, oh * 128:(oh + 1) * 128]
    return out


def _build_br(half):
    """[68, 128]: cols rt*64+hl; rows are shard-local rows within row-tile rt."""
    hbase = half * HH
    M = np.zeros((68, 128), dtype=np.float32)
    for rt in range(2):
        for hl in range(64):
            hg = hbase + rt * 64 + hl
            for i in range(5):
                r = _reflect_idx(hg - 2 + i, H)
                j = r + 2 - hbase          # canonical shard row
                M[j - rt * 64, rt * 64 + hl] += 1.0
    return M


def _ksegs_in_octet(oct_idx):
    """Pair channels live at ch 16..151 (pr k-major). For octet [oct*8, +8),
    return (j0, k, l0, nl): local offset j0, channel k, first l, count."""
    lo, hi = oct_idx * 8, oct_idx * 8 + 8
    segs = []
    p = 0
    for k in range(K):
        n = K - k
        s, e = 16 + p, 16 + p + n
        a, b = max(lo, s), min(hi, e)
        if a < b:
            segs.append((a - lo, k, k + (a - s), b - a))
        p += n
    return segs


def _pr0(k):
    """pr index of pair (k, k)."""
    return k * K - (k * (k - 1)) // 2


def _build_kernel():
    nc = bacc.Bacc("TRN2", target_bir_lowering=False, debug=False)
    S_d = nc.dram_tensor("S", [SR, K, W], BF16, kind="ExternalInput").ap()
    BR_d = nc.dram_tensor("BR", [68, 128], BF16, kind="ExternalInput").ap()
    BW_d = nc.dram_tensor("BW", [128, 512], BF16, kind="ExternalInput").ap()
    # output: upper triangle only, [w, rt, pr, hh] bf16
    R_d = nc.dram_tensor("R", [W, 2, NP, 64], BF16, kind="ExternalOutput").ap()

    with tile.TileContext(nc) as tc, ExitStack() as ctx:
        const_p = ctx.enter_context(tc.tile_pool(name="const", bufs=1))
        sp_p = ctx.enter_context(tc.tile_pool(name="sp", bufs=1))
        t_p = ctx.enter_context(tc.tile_pool(name="tprod", bufs=3))
        i1_p = ctx.enter_context(tc.tile_pool(name="i1", bufs=2))
        mu_p = ctx.enter_context(tc.tile_pool(name="mu", bufs=2))
        m_p = ctx.enter_context(tc.tile_pool(name="mm", bufs=2))
        r_p = ctx.enter_context(tc.tile_pool(name="rout", bufs=2))
        e2_p = ctx.enter_context(tc.tile_pool(name="e2", bufs=3))
        ps1_p = ctx.enter_context(tc.tile_pool(name="ps1", bufs=2, space="PSUM"))
        ps2_p = ctx.enter_context(tc.tile_pool(name="ps2", bufs=4, space="PSUM"))

        br = const_p.tile([68, 128], BF16)
        bw = const_p.tile([128, 512], BF16)
        nc.sync.dma_start(br[:], BR_d)
        nc.sync.dma_start(bw[:], BW_d)

        sp0 = sp_p.tile([68, K, W], BF16)
        sp1 = sp_p.tile([68, K, W], BF16)
        nc.sync.dma_start(sp0[:], S_d[0:68])
        nc.sync.dma_start(sp1[:], S_d[64:132])
        sps = [sp0, sp1]

        for rt in range(2):
            sp = sps[rt]
            brt = br[:, rt * 64:(rt + 1) * 64]
            # ---------------- stage 1: products + H-box ----------------
            # i1 layout: [w(128), wchunk(2), ch(NCH), h(64)]
            i1 = i1_p.tile([128, 2, NCH, 64], BF16, name="i1")
            for oc in range(NOCT):
                if oc < 2:   # mean channels read straight from sp
                    T = sp[:, oc * 8:(oc + 1) * 8, :]
                else:
                    Tt = t_p.tile([68, 8, W], BF16, name="T")
                    for (j0, k, l0, nl) in _ksegs_in_octet(oc):
                        in0 = sp[:, k, :].unsqueeze(1).broadcast_to([68, nl, W])
                        nc.vector.tensor_mul(
                            Tt[:, j0:j0 + nl, :], in0, sp[:, l0:l0 + nl, :])
                    T = Tt[:]
                ps1 = ps1_p.tile([128, 2, 8, 64], F32, name="ps1")
                for j in range(8):
                    nc.tensor.matmul(ps1[:, 0, j, :],
                                     T[:, j, 0:128], brt,
                                     start=True, stop=True)
                    nc.tensor.matmul(ps1[:, 1, j, :],
                                     T[:, j, 128:256], brt,
                                     start=True, stop=True)
                nc.scalar.copy(i1[:, :, oc * 8:(oc + 1) * 8, :], ps1[:])

            # ---------------- stage 2: W-box + finish ----------------
            for oh in range(2):
                bwa = bw[:, (oh * 2) * 128:(oh * 2) * 128 + 128]
                bwb = bw[:, (oh * 2 + 1) * 128:(oh * 2 + 1) * 128 + 128]
                # --- mean channels -> mu (bf16, scaled by 1/5) ---
                mub = mu_p.tile([128, K, 64], BF16, name="mub")
                for mo in range(2):
                    psm = ps2_p.tile([128, 8, 64], F32, name="ps2")
                    nc.tensor.matmul(psm[:], bwa,
                                     i1[:, 0, mo * 8:(mo + 1) * 8, :],
                                     start=True, stop=False)
                    nc.tensor.matmul(psm[:], bwb,
                                     i1[:, 1, mo * 8:(mo + 1) * 8, :],
                                     start=False, stop=True)
                    nc.scalar.mul(mub[:, mo * 8:(mo + 1) * 8, :], psm[:], 0.2)
                # --- M = mu_k * mu_l (pr-major), DVE/GpSimd split ---
                M = m_p.tile([128, NP, 64], BF16, name="M")
                for k in range(K):
                    nl = K - k
                    p0 = _pr0(k)
                    in0 = mub[:, k, :].unsqueeze(1).broadcast_to([128, nl, 64])
                    eng = nc.gpsimd if k < C_POOL_KMAX else nc.vector
                    eng.tensor_mul(M[:, p0:p0 + nl, :], in0, mub[:, k:K, :])
                # --- pair channels: W-box, subtract, store triangle ---
                rsb = r_p.tile([128, NP, 64], BF16, name="rsb")
                for oc in range(2, NOCT):
                    ps2 = ps2_p.tile([128, 8, 64], F32, name="ps2")
                    nc.tensor.matmul(ps2[:], bwa,
                                     i1[:, 0, oc * 8:(oc + 1) * 8, :],
                                     start=True, stop=False)
                    nc.tensor.matmul(ps2[:], bwb,
                                     i1[:, 1, oc * 8:(oc + 1) * 8, :],
                                     start=False, stop=True)
                    pr = (oc - 2) * 8
                    gunit = (rt * 2 + oh) * 17 + (oc - 2)
                    if gunit % D_POOL_FRAC == D_POOL_FRAC - 1:
                        e2 = e2_p.tile([128, 8, 64], BF16, name="e2")
                        nc.scalar.copy(e2[:], ps2[:])
                        nc.gpsimd.tensor_sub(rsb[:, pr:pr + 8, :], e2[:],
                                             M[:, pr:pr + 8, :])
                    else:
                        nc.vector.tensor_sub(rsb[:, pr:pr + 8, :], ps2[:],
                                             M[:, pr:pr + 8, :])
                # DMA out: R[w-half oh, rt, :, :]
                nc.sync.dma_start(R_d[oh * 128:(oh + 1) * 128, rt], rsb[:])

    nc.compile()
    return nc


_NC_CACHE = {}


def _get_nc():
    if "nc" not in _NC_CACHE:
        _NC_CACHE["nc"] = _build_kernel()
    return _NC_CACHE["nc"]


def _prep_in_maps(S):
    S = np.asarray(S, dtype=np.float32)
    np_bf16 = mybir.dt.np(BF16)
    bw = _build_bw().astype(np_bf16)
    brs = [(_build_br(h)).astype(np_bf16) for h in range(2)]
    Ss = S * np.float32(0.2)
    in_maps = []
    for b in range(B):
        for half in range(2):
            hbase = half * HH
            rows = np.clip(np.arange(hbase - 2, hbase + 130), 0, H - 1)
            shard = Ss[b][:, rows, :].transpose(1, 0, 2)   # [132, K, 256]
            shard = np.ascontiguousarray(shard).astype(np_bf16)
            in_maps.append({"S": shard, "BR": brs[half], "BW": bw})
    return in_maps


# upper-tri gather index: IU[k, l] = pr for (min,max)
_IU = np.zeros((K, K), dtype=np.int64)
for _k in range(K):
    for _l in range(_k, K):
        _IU[_k, _l] = _IU[_l, _k] = _pr0(_k) + (_l - _k)


def _assemble(results):
    out = np.empty((B, H, W, K, K), dtype=np.float32)
    for i in range(8):
        b, half = divmod(i, 2)
        rd = np.asarray(results[i]["R"]).astype(np.float32)  # [256, 2, 136, 64]
        tri = rd.transpose(1, 3, 0, 2).reshape(HH, W, NP)    # [h, w, pr]
        out[b, half * HH:(half + 1) * HH] = tri[:, :, _IU]
    return out


def kernel(S):
    """S: [4, 16, 256, 256] float32 -> R: [4, 256, 256, 16, 16] float32."""
    nc = _get_nc()
    in_maps = _prep_in_maps(S)
    res = bass_utils.run_bass_kernel_spmd(nc, in_maps, list(range(8)))
    return _assemble(res.results)
